# revision 1
# baseline (speedup 1.0000x reference)
"""Linear attention ("Transformers are RNNs") on 8 Trainium2 NeuronCores.

Problem: N=8, L=S=8192, H=8, D=Dv=32, f32.
    phi(x) = elu(x)+1
    A[d,v] = sum_s phi(K)[s,d] V[s,v]        (the /v_length ... *v_length cancels exactly)
    b[d]   = sum_s phi(K)[s,d]
    out[l,v] = (sum_d phi(Q)[l,d] A[d,v]) / (sum_d phi(Q)[l,d] b[d] + EPS)

Sharding: batch element n -> core n (fully independent, no collectives).

Device design (final):
  - bf16 compute throughout (rel err ~2.6e-3 vs the f32 reference; the
    harness gate is 2e-2): inputs are cast to bf16 on the host, halving
    DMA traffic.  PSUM accumulation, the denominator and the reciprocal
    stay f32.  Output is bf16 on-device, cast back to f32 on the host.
  - Q pre-transposed on host to [H*D, L]: contraction dim d lands on SBUF
    partitions with fully contiguous DMA; no on-device transposes.
  - V is sent as [S, 258] = [V_g0 | 1 | V_g1 | 1]: the ones column folds
    the b = sum_s phi(K) accumulation into the same matmul as A.
  - phi(x) = min(exp(x), 1 + relu(x))  (exactly elu(x)+1):
    e = Exp(x) (ScalarE); t = (x max 0)+1 (VectorE dual-op tensor_scalar,
    4x mode); phi = min(e, t) (VectorE tensor_tensor, 2x mode).
  - A 9-matmul N=512 dummy burst at kernel start warms the PE clock gate
    (HAM) to 2.4 GHz while the first DMAs prefill.
  - Phase 1 (64 s-subtiles of 128 in macros of 8): per 4-head group one
    bf16 matmul  lhsT = phi(K)_g [s=128, (j,d)=128], rhs = [V_g | 1]
    (N=129), accumulated over all of S into PSUM[128, 129] per group.
    Diagonal 32x32 j-blocks are A_h; col 128 is b_h.
  - Phase 1.5: assemble per group: block-diag A [128,128] bf16 and
    block-diag b columns [128,4] bf16.
  - Phase 2 (64 l-subtiles in macros of 4; 8 of 16 macros'
    DMA+phi(Q) are interleaved into the phase-1 loop): per group two
    matmuls share the same stationary phi(Q)^T slice: numer (N=128,
    lands directly in the output layout) and den (N=4, batched per macro
    into one PSUM bank so one reciprocal serves 4 subtiles).  EPS is
    dropped: den ~ 2e5, so EPS=1e-6 is a 1e-11 relative perturbation,
    far below bf16 rounding.  Normalize with one broadcast
    tensor_tensor multiply per 2 subtiles; DMA out [l, h*32+v] bf16.

Host sends K and V in macro-tiled linear layouts [n_macro, 128, cols] so
each phase-1 DMA is one fully contiguous block (4KB/2KB packets instead of
~512B runs).  Phase-2 Q-prep (DMA + phi) for 8 of 16 macros is interleaved
into the phase-1 loop.  Measured on 8 NeuronCores: HW exec 86-99 us
across runs (median ~92; identical binary, run-to-run variance from HAM
clock-gate phase and shared-chip HBM), rel err 2.6e-3.
"""

import sys

for _p in ("/opt/trn_rl_repo",):
    if _p not in sys.path:
        sys.path.insert(0, _p)

import ml_dtypes
import numpy as np

from concourse import bacc, bass, mybir, tile
from concourse.bass_utils import run_bass_kernel_spmd

# ---------------------------------------------------------------- constants
N_BATCH = 8
L = 8192
S = 8192
H = 8
D = 32
HD = H * D  # 256
P = 128
EPS = 1e-6

F32 = mybir.dt.float32
BF16 = mybir.dt.bfloat16
FP8 = mybir.dt.float8e4
AF = mybir.ActivationFunctionType
OP = mybir.AluOpType

MACRO = 8  # 128-row s-subtiles per phase-1 macro tile
N_MACRO = S // (P * MACRO)  # 8
QMACRO = 4  # l-subtiles per phase-2 macro
N_QMACRO = L // (P * QMACRO)  # 16

G = 2  # head groups (4 heads each)
VA = P + 1  # 129: V group columns + ones column
VR = G * VA  # 258: host-side V row: [V_g0 | 1 | V_g1 | 1]


def _bcast_last(ap, n):
    """Append a stride-0 dim of size n to an AP (free-dim broadcast)."""
    ap = ap.unsqueeze(ap.ndim)
    return ap.broadcast_to(tuple(ap.shape[:-1]) + (n,))


def _phi(nc, pool, x, fd, pfx="", obufs=None):
    """phi(x) = elu(x)+1 = min(exp(x), 1 + relu(x)); x is [P, fd] bf16 SBUF."""
    e = pool.tile([P, fd], BF16, tag=pfx + "phi_e", name=pfx + "phi_e")
    t = pool.tile([P, fd], BF16, tag=pfx + "phi_t", name=pfx + "phi_t")
    kw = {"bufs": obufs} if obufs else {}
    phi = pool.tile([P, fd], BF16, tag=pfx + "phi_o", name=pfx + "phi_o", **kw)
    nc.scalar.activation(e[:], x[:], AF.Exp)
    nc.vector.tensor_scalar(t[:], x[:], 0.0, 1.0, OP.max, OP.add)
    nc.vector.tensor_tensor(phi[:], e[:], t[:], OP.min)
    return phi


def _phi2(nc, pool, x, fd):
    """phi = (exp(x) min 1) + relu(x); exp and relu on ScalarE, one DVE
    scalar_tensor_tensor combines them (rebalances DVE -> ACT)."""
    e = pool.tile([P, fd], BF16, tag="phi_e")
    r = pool.tile([P, fd], BF16, tag="phi_r")
    phi = pool.tile([P, fd], BF16, tag="phi_o")
    nc.scalar.activation(e[:], x[:], AF.Exp)
    nc.scalar.activation(r[:], x[:], AF.Relu)
    nc.vector.scalar_tensor_tensor(phi[:], e[:], 1.0, r[:], OP.min, OP.add)
    return phi


def _build_body(nc, tc, qt, kk, vv, out):
    with (
        tc.tile_pool(name="io", bufs=4, ) as io,
        tc.tile_pool(name="ew", bufs=3) as ew,
        tc.tile_pool(name="ew2", bufs=18) as ew2,
        tc.tile_pool(name="misc", bufs=1) as misc,
        tc.tile_pool(name="small", bufs=3) as small,
        tc.tile_pool(name="outp", bufs=4) as outp,
    ):
        def _qprep(mq):
            c0 = mq * QMACRO * P
            ph = []
            for g in range(G):
                qt_t = io.tile([P, QMACRO * P], BF16, tag=f"qt{g}", name=f"qt{g}")
                nc.sync.dma_start(
                    qt_t[:], qt[g * P : (g + 1) * P, c0 : c0 + QMACRO * P]
                )
                ph.append(_phi(nc, ew2, qt_t, QMACRO * P, pfx="q"))
            return ph

        pre_phis = {}

        # ---------------- phase 1: A/b accumulation over S ----------------
        with tc.tile_pool(name="ps1", bufs=1, space="PSUM") as ps1:
            pacc = [
                ps1.tile([P, VA], F32, tag=f"pacc{g}", name=f"pacc{g}")
                for g in range(G)
            ]

            # HAM warm-up: a dense dummy matmul burst while the initial DMAs
            # prefill.  ~16 N=512 matmuls = ~5us of continuous PE activity
            # flips the clock gate to 8/8 (2.4 GHz); the real MM stream then
            # never idles long enough (>3.4us) to re-throttle.
            wz = misc.tile([P, 512], BF16, tag="warm", name="warm")
            nc.vector.memset(wz[:], 0.0)
            junk = ps1.tile([P, 512], F32, tag="junk", name="junk")
            for _ in range(9):
                nc.tensor.matmul(
                    junk[:], wz[:, 0:P], wz[:], start=True, stop=True
                )

            for m in range(N_MACRO):
                k_t = io.tile([P, MACRO * HD], BF16, tag="k_t")
                nc.sync.dma_start(k_t[:], kk[m])
                v_t = io.tile([P, MACRO * VR], BF16, tag="v_t")
                nc.sync.dma_start(v_t[:], vv[m])

                phi = _phi(nc, ew, k_t, MACRO * HD)

                first = m == 0
                last = m == N_MACRO - 1
                for b in range(MACRO):
                    for g in range(G):
                        nc.tensor.matmul(
                            pacc[g][:],
                            phi[:, b * HD + g * P : b * HD + (g + 1) * P],
                            v_t[:, b * VR + g * VA : b * VR + (g + 1) * VA],
                            start=(first and b == 0),
                            stop=(last and b == MACRO - 1),
                        )

                pre_phis[m] = _qprep(m)

            # ------------- phase 1.5: block-diag A, block-diag b ----------
            amat = []
            bmat = []
            for g in range(G):
                ag = misc.tile([P, P], BF16, tag=f"amat{g}", name=f"amat{g}")
                bg = misc.tile([P, 4], BF16, tag=f"bmat{g}", name=f"bmat{g}")
                nc.vector.memset(ag[:], 0.0)
                nc.vector.memset(bg[:], 0.0)
                for j in range(4):
                    r0 = 32 * j
                    nc.scalar.copy(
                        ag[r0 : r0 + 32, r0 : r0 + 32],
                        pacc[g][r0 : r0 + 32, r0 : r0 + 32],
                    )
                    nc.scalar.copy(
                        bg[r0 : r0 + 32, j : j + 1],
                        pacc[g][r0 : r0 + 32, P : P + 1],
                    )
                amat.append(ag)
                bmat.append(bg)

            # keep PE warm across the phase-1.5 transition
            for _ in range(6):
                nc.tensor.matmul(
                    junk[:], wz[:, 0:P], wz[:], start=True, stop=True
                )

        # ---------------- phase 2: queries ----------------
        with (
            tc.tile_pool(name="ps2n", bufs=5, space="PSUM") as ps2n,
            tc.tile_pool(name="ps2d", bufs=3, space="PSUM") as ps2d,
        ):
            for mq in range(N_QMACRO):
                c0 = mq * QMACRO * P
                phis = pre_phis.get(mq) or _qprep(mq)

                # den PSUM for the whole macro: cols (sub, g, j)
                den_ps = ps2d.tile([P, QMACRO * G * 4], F32, tag="den_ps")
                numers = []
                nm = None
                for i in range(QMACRO):
                    if i % 2 == 0:
                        nm = ps2n.tile([P, 2 * HD], F32, tag="nm")
                        numers.append(nm)
                    for g in range(G):
                        w = phis[g][:, i * P : (i + 1) * P]
                        nc.tensor.matmul(
                            nm[:, (i % 2) * HD + g * P : (i % 2) * HD + (g + 1) * P],
                            w,
                            amat[g][:],
                            start=True,
                            stop=True,
                        )
                        nc.tensor.matmul(
                            den_ps[:, (i * G + g) * 4 : (i * G + g + 1) * 4],
                            w,
                            bmat[g][:],
                            start=True,
                            stop=True,
                        )

                rcp = small.tile([P, QMACRO * G * 4], F32, tag="rcp")
                nc.vector.reciprocal(rcp[:], den_ps[:])

                for pr in range(QMACRO // 2):
                    out_t = outp.tile([P, 2 * HD], BF16, tag="out_t")
                    rv = rcp[:, 2 * pr * G * 4 : (2 * pr + 2) * G * 4].rearrange(
                        "p (s g j) -> p s g j", s=2, g=G
                    )
                    nc.vector.tensor_tensor(
                        out_t[:].rearrange(
                            "p (s g j c) -> p s g j c", s=2, g=G, c=32
                        ),
                        numers[pr][:].rearrange(
                            "p (s g j c) -> p s g j c", s=2, g=G, c=32
                        ),
                        _bcast_last(rv, 32),
                        OP.mult,
                    )
                    r0 = c0 + 2 * pr * P
                    nc.sync.dma_start(
                        out[r0 : r0 + 2 * P, :].rearrange("(s p) c -> p s c", p=P),
                        out_t[:].rearrange("p (s c) -> p s c", s=2),
                    )


_NC_CACHE = None


def build_nc():
    global _NC_CACHE
    if _NC_CACHE is not None:
        return _NC_CACHE
    nc = bacc.Bacc(
        "TRN2",
        target_bir_lowering=False,
        debug=False,
        enable_asserts=False,
        num_devices=N_BATCH,
    )
    qt = nc.dram_tensor("qt", [HD, L], BF16, kind="ExternalInput").ap()
    kk = nc.dram_tensor("kk", [N_MACRO, P, MACRO * HD], BF16, kind="ExternalInput").ap()
    vv = nc.dram_tensor("vv", [N_MACRO, P, MACRO * VR], BF16, kind="ExternalInput").ap()
    out = nc.dram_tensor("out", [L, HD], BF16, kind="ExternalOutput").ap()
    with tile.TileContext(nc) as tc:
        _build_body(nc, tc, qt, kk, vv, out)
    nc.compile()
    return nc


def make_in_maps(queries, keys, values):
    queries = np.asarray(queries, dtype=np.float32)
    keys = np.asarray(keys, dtype=np.float32)
    values = np.asarray(values, dtype=np.float32)
    bf = ml_dtypes.bfloat16
    in_maps = []
    for n in range(N_BATCH):
        v2 = values[n].reshape(S, HD)
        vva = np.ones((S, VR), dtype=bf)
        vva[:, 0:P] = v2[:, 0:P].astype(bf)
        vva[:, VA : VA + P] = v2[:, P : 2 * P].astype(bf)
        # macro-tiled linear layouts: [m, p, b*cols+c] so each macro DMA is
        # one fully contiguous block
        kmac = np.ascontiguousarray(
            keys[n].reshape(N_MACRO, MACRO, P, HD).transpose(0, 2, 1, 3)
            .reshape(N_MACRO, P, MACRO * HD).astype(bf))
        vmac = np.ascontiguousarray(
            vva.reshape(N_MACRO, MACRO, P, VR).transpose(0, 2, 1, 3)
            .reshape(N_MACRO, P, MACRO * VR))
        qt = np.ascontiguousarray(
            queries[n].transpose(1, 2, 0).reshape(HD, L).astype(bf)
        )  # [h*32+d, l]
        in_maps.append(
            {
                "qt": qt,
                "kk": kmac,
                "vv": vmac,
            }
        )
    return in_maps


def run(queries, keys, values, trace=False, **kwargs):
    nc = build_nc()
    in_maps = make_in_maps(queries, keys, values)
    res = run_bass_kernel_spmd(
        nc, in_maps, core_ids=list(range(N_BATCH)), trace=trace, **kwargs
    )
    outs = [
        res.results[n]["out"].astype(np.float32).reshape(L, H, D)
        for n in range(N_BATCH)
    ]
    return np.stack(outs, axis=0), res


def kernel(queries, keys, values):
    out, _ = run(queries, keys, values, trace=False)
    return out



# revision 4
# speedup vs baseline: 1.0621x; 1.0621x over previous
"""Linear attention ("Transformers are RNNs") on 8 Trainium2 NeuronCores.

Problem: N=8, L=S=8192, H=8, D=Dv=32, f32.
    phi(x) = elu(x)+1 = min(exp(x), 1+relu(x))
    A[d,v] = sum_s phi(K)[s,d] V[s,v]     (the /v_length ... *v_length cancels)
    b[d]   = sum_s phi(K)[s,d]
    out[l,v] = (sum_d phi(Q)[l,d] A[d,v]) / (sum_d phi(Q)[l,d] b[d] + EPS)

Sharding: batch element n -> core n (fully independent, no collectives).

v2 design — single continuous DMA-bound stream, group-pipelined:
  - Heads split into G=2 groups of 4 (linear attention is separable per
    head).  K/V stream group-major, so group 0's A/b finish at the half-way
    point of the input stream and group 0's entire query pass (matmuls,
    reciprocal, normalize, output DMA) overlaps group 1's K/V accumulation.
    Only group 1's query pass sits in the tail (~1/4 of the work).
  - All DMAs are large fully-contiguous slabs ([128, 2048+] per transfer):
    16x K|V slabs (516KB), 8x Q (512KB), 8x out (512KB).  Input DMAs are
    issued on the sync queue in stream order; output DMAs go on the gpsimd
    queue so a not-yet-ready output never head-of-line-blocks the input
    stream.
  - Engine balance (errata-adjusted cost model):
      ACT:    exp(K), exp(Q), A/b assembly copies           (~35us)
      DVE:    (max,+1) for K and Q (4x), min for Q (2x),
              batched reciprocal_approx_fast, normalize TT  (~41us)
      GPSIMD: min for K, output DMA issue                   (~26us)
      PE:     A-accumulation MMs, numer (N=128) + den (N=4) (~30us)
      DMA:    16.9 MB bf16 at ~390 GB/s                     (~43us)
  - Normalize: one tensor_tensor per 2 q-macros over a 2-bank PSUM tile
    [128, 1024] with a stride-0 broadcast reciprocal operand (1x mode is
    forced by the f32 PSUM read anyway, so nothing is lost to broadcast).
  - Reciprocal via reciprocal_approx_fast (~18 correct bits, den ~1e5 so
    EPS=1e-6 is a 1e-11 perturbation and is dropped).
  - Junk-MM bursts at kernel start and at the two group barriers keep the
    PE HAM clock gate at 8/8 where it matters.
"""

import sys

for _p in ("/opt/trn_rl_repo",):
    if _p not in sys.path:
        sys.path.insert(0, _p)

import ml_dtypes
import numpy as np

from concourse import bacc, bass, mybir, tile
from concourse.bass_utils import run_bass_kernel_spmd

# ---------------------------------------------------------------- constants
N_BATCH = 8
L = 8192
S = 8192
H = 8
D = 32
P = 128

F32 = mybir.dt.float32
BF16 = mybir.dt.bfloat16
AF = mybir.ActivationFunctionType
OP = mybir.AluOpType

G = 2          # head groups (4 heads each; 4*32 = 128 partitions)
NM = 8         # K/V s-macros per group (1024 s-rows each)
MB = 8         # 128-row s-subtiles per macro
VA = P + 1     # 129: V group columns + ones column
KCOLS = MB * P         # 1024
VCOLS = MB * VA        # 1032
KVCOLS = KCOLS + VCOLS  # 2056
NDP = 4        # Q double-pairs per group (2048 l-columns each)
QCOLS = 2048


def _bcast_last(ap, n):
    """Append a stride-0 dim of size n to an AP (free-dim broadcast)."""
    ap = ap.unsqueeze(ap.ndim)
    return ap.broadcast_to(tuple(ap.shape[:-1]) + (n,))


def _build_body(nc, tc, qq, kv, og):
    with (
        tc.tile_pool(name="iokv", bufs=3) as iokv,
        tc.tile_pool(name="ioq", bufs=2) as ioq,
        tc.tile_pool(name="ewk", bufs=2) as ewk,
        tc.tile_pool(name="ewq", bufs=2) as ewq,
        tc.tile_pool(name="qp", bufs=1) as qp,
        tc.tile_pool(name="misc", bufs=1) as misc,
        tc.tile_pool(name="small", bufs=2) as small,
        tc.tile_pool(name="outp", bufs=2) as outp,
        tc.tile_pool(name="pacc", bufs=1, space="PSUM") as paccp,
        tc.tile_pool(name="psn", bufs=2, space="PSUM") as psn,
        tc.tile_pool(name="psd", bufs=1, space="PSUM") as psd,
    ):
        pacc = [
            paccp.tile([P, 512], F32, tag=f"pacc{g}", name=f"pacc{g}")
            for g in range(G)
        ]
        phiq = {}
        amat = {}
        bmat = {}

        # HAM warm-up: dense dummy matmuls while the first DMAs prefill.
        wz = misc.tile([P, 512], BF16, tag="warm", name="warm")
        nc.vector.memset(wz[:], 0.0)
        for _ in range(9):
            nc.tensor.matmul(
                pacc[0][:], wz[:, 0:P], wz[:], start=True, stop=True
            )

        def a_macro(g, m):
            kvt = iokv.tile([P, KVCOLS], BF16, tag="kv")
            nc.sync.dma_start(kvt[:], kv[g, m])
            kpart = kvt[:, 0:KCOLS]
            e = ewk.tile([P, KCOLS], BF16, tag="ke")
            t = ewk.tile([P, KCOLS], BF16, tag="kt")
            ph = ewk.tile([P, KCOLS], BF16, tag="kphi")
            nc.scalar.activation(e[:], kpart, AF.Exp)
            nc.vector.tensor_scalar(t[:], kpart, 0.0, 1.0, OP.max, OP.add)
            nc.vector.tensor_tensor(ph[:], e[:], t[:], OP.min)
            for b in range(MB):
                nc.tensor.matmul(
                    pacc[g][:, 0:VA],
                    ph[:, b * P : (b + 1) * P],
                    kvt[:, KCOLS + b * VA : KCOLS + (b + 1) * VA],
                    start=(m == 0 and b == 0),
                    stop=(m == NM - 1 and b == MB - 1),
                )

        def qprep(g, dp):
            qt = ioq.tile([P, QCOLS], BF16, tag="qt")
            nc.sync.dma_start(qt[:], qq[g, dp])
            e = ewq.tile([P, QCOLS], BF16, tag="qe")
            t = ewq.tile([P, QCOLS], BF16, tag="qt2")
            ph = qp.tile([P, QCOLS], BF16, tag=f"phiq{g}_{dp}",
                         name=f"phiq{g}_{dp}")
            nc.scalar.activation(e[:], qt[:], AF.Exp)
            nc.vector.tensor_scalar(t[:], qt[:], 0.0, 1.0, OP.max, OP.add)
            nc.vector.tensor_tensor(ph[:], e[:], t[:], OP.min)
            phiq[(g, dp)] = ph

        def assemble(g):
            am = misc.tile([P, P], BF16, tag=f"am{g}", name=f"am{g}")
            bm = misc.tile([P, 4], BF16, tag=f"bm{g}", name=f"bm{g}")
            nc.vector.memset(am[:], 0.0)
            nc.vector.memset(bm[:], 0.0)
            for j in range(4):
                r0 = 32 * j
                nc.scalar.copy(
                    am[r0 : r0 + 32, r0 : r0 + 32],
                    pacc[g][r0 : r0 + 32, r0 : r0 + 32],
                )
                nc.scalar.copy(
                    bm[r0 : r0 + 32, j : j + 1],
                    pacc[g][r0 : r0 + 32, P : P + 1],
                )
            amat[g] = am
            bmat[g] = bm
            # keep the PE warm across the barrier
            for _ in range(4):
                nc.tensor.matmul(
                    pacc[g][:], wz[:, 0:P], wz[:], start=True, stop=True
                )

        # state shared across a double-pair (two b_pair calls)
        dpstate = {}

        def b_pair(g, mp):
            """Query pass for one pair of q-macros (1024 l-rows)."""
            half = mp % 2
            if half == 0:
                dpstate["dn"] = psd.tile([P, 64], F32, tag="dn", name="dn")
                dpstate["ot"] = outp.tile([P, 2 * 1024], BF16, tag="ot", name="ot")
            dn = dpstate["dn"]
            ot = dpstate["ot"]
            nm = psn.tile([P, 1024], F32, tag="nm")
            ph = phiq[(g, mp // 2)]
            for qs in range(8):  # (qmacro-in-pair, subtile)
                w = ph[:, (half * 8 + qs) * P : (half * 8 + qs + 1) * P]
                nc.tensor.matmul(
                    nm[:, qs * P : (qs + 1) * P], w, amat[g][:],
                    start=True, stop=True,
                )
                nc.tensor.matmul(
                    dn[:, half * 32 + qs * 4 : half * 32 + (qs + 1) * 4],
                    w, bmat[g][:], start=True, stop=True,
                )
            rcp = small.tile([P, 32], F32, tag="rcp")
            nc.vector.reciprocal_approx_fast(
                out=rcp[:], in_=dn[:, half * 32 : half * 32 + 32]
            )
            nc.vector.tensor_tensor(
                ot[:, half * 1024 : (half + 1) * 1024].rearrange(
                    "p (qs j c) -> p qs j c", qs=8, j=4, c=32
                ),
                nm[:].rearrange("p (qs j c) -> p qs j c", qs=8, j=4, c=32),
                _bcast_last(
                    rcp[:].rearrange("p (qs j) -> p qs j", qs=8, j=4), 32
                ),
                OP.mult,
            )
            if half == 1:
                # output DMA on the gpsimd queue: never blocks input stream
                nc.gpsimd.dma_start(og[g, mp // 2], ot[:])

        # ---------------- group 0: A/b accumulation + Q prep ----------------
        for m in range(NM):
            a_macro(0, m)
            if m % 2 == 0:
                qprep(0, m // 2)
        assemble(0)

        # -------- group 1 accumulation overlapped with group 0 queries ------
        for m in range(NM):
            a_macro(1, m)
            if m % 2 == 0:
                qprep(1, m // 2)
            b_pair(0, m)
        assemble(1)

        # ---------------- group 1 queries (tail) ----------------
        for mp in range(2 * NDP):
            b_pair(1, mp)


_NC_CACHE = None


def build_nc():
    global _NC_CACHE
    if _NC_CACHE is not None:
        return _NC_CACHE
    nc = bacc.Bacc(
        "TRN2",
        target_bir_lowering=False,
        debug=False,
        enable_asserts=False,
        num_devices=N_BATCH,
    )
    qq = nc.dram_tensor("qq", [G, NDP, P, QCOLS], BF16, kind="ExternalInput").ap()
    kv = nc.dram_tensor("kv", [G, NM, P, KVCOLS], BF16, kind="ExternalInput").ap()
    og = nc.dram_tensor("og", [G, NDP, P, 2 * 1024], BF16, kind="ExternalOutput").ap()
    with tile.TileContext(nc) as tc:
        _build_body(nc, tc, qq, kv, og)
    nc.compile()
    _NC_CACHE = nc
    return nc


def make_in_maps(queries, keys, values):
    queries = np.asarray(queries, dtype=np.float32)
    keys = np.asarray(keys, dtype=np.float32)
    values = np.asarray(values, dtype=np.float32)
    bf = ml_dtypes.bfloat16
    in_maps = []
    for n in range(N_BATCH):
        kvn = np.empty((G, NM, P, KVCOLS), dtype=bf)
        qqn = np.empty((G, NDP, P, QCOLS), dtype=bf)
        for g in range(G):
            # K group slab: [m][p][(b, jd)]
            Kg = keys[n][:, 4 * g : 4 * g + 4, :].reshape(S, P)
            kvn[g, :, :, 0:KCOLS] = (
                Kg.reshape(NM, MB, P, P).transpose(0, 2, 1, 3)
                .reshape(NM, P, KCOLS).astype(bf)
            )
            # V group slab with ones column: [m][p][(b, v|1)]
            Vg = values[n][:, 4 * g : 4 * g + 4, :].reshape(S, P)
            V1 = np.ones((S, VA), dtype=np.float32)
            V1[:, 0:P] = Vg
            kvn[g, :, :, KCOLS:] = (
                V1.reshape(NM, MB, P, VA).transpose(0, 2, 1, 3)
                .reshape(NM, P, VCOLS).astype(bf)
            )
            # Q transposed group-major: [dp][jd, l]
            Qg = queries[n][:, 4 * g : 4 * g + 4, :].reshape(L, P)
            qqn[g] = (
                Qg.T.reshape(P, NDP, QCOLS).transpose(1, 0, 2).astype(bf)
            )
        in_maps.append({"qq": qqn, "kv": kvn})
    return in_maps


def run(queries, keys, values, trace=False, **kwargs):
    nc = build_nc()
    in_maps = make_in_maps(queries, keys, values)
    res = run_bass_kernel_spmd(
        nc, in_maps, core_ids=list(range(N_BATCH)), trace=trace, **kwargs
    )
    outs = []
    for n in range(N_BATCH):
        o = res.results[n]["og"].astype(np.float32)
        # og[g, dp, p, (mp2, q, s, j, v)]; l = (((dp*2+mp2)*2+q)*4+s)*128+p
        o = o.reshape(G, NDP, P, 2, 2, 4, 4, 32)
        o = o.transpose(1, 3, 4, 5, 2, 0, 6, 7).reshape(L, H, D)
        outs.append(o)
    return np.stack(outs, axis=0), res


def kernel(queries, keys, values):
    out, _ = run(queries, keys, values, trace=False)
    return out


# revision 5
# speedup vs baseline: 1.0693x; 1.0067x over previous
"""Linear attention ("Transformers are RNNs") on 8 Trainium2 NeuronCores.

Problem: N=8, L=S=8192, H=8, D=Dv=32, f32.
    phi(x) = elu(x)+1 = min(exp(x), 1+relu(x))
    A[d,v] = sum_s phi(K)[s,d] V[s,v]     (the /v_length ... *v_length cancels)
    b[d]   = sum_s phi(K)[s,d]
    out[l,v] = (sum_d phi(Q)[l,d] A[d,v]) / (sum_d phi(Q)[l,d] b[d] + EPS)

Sharding: batch element n -> core n (fully independent, no collectives).

v2 design — single continuous DMA-bound stream, group-pipelined:
  - Heads split into G=2 groups of 4 (linear attention is separable per
    head).  K/V stream group-major, so group 0's A/b finish at the half-way
    point of the input stream and group 0's entire query pass (matmuls,
    reciprocal, normalize, output DMA) overlaps group 1's K/V accumulation.
    Only group 1's query pass sits in the tail (~1/4 of the work).
  - All DMAs are large fully-contiguous slabs ([128, 2048+] per transfer):
    16x K|V slabs (516KB), 8x Q (512KB), 8x out (512KB).  Input DMAs are
    issued on the sync queue in stream order; output DMAs go on the gpsimd
    queue so a not-yet-ready output never head-of-line-blocks the input
    stream.
  - Engine balance (errata-adjusted cost model):
      ACT:    exp(K), exp(Q), A/b assembly copies           (~35us)
      DVE:    (max,+1) for K and Q (4x), min for Q (2x),
              batched reciprocal_approx_fast, normalize TT  (~41us)
      GPSIMD: min for K, output DMA issue                   (~26us)
      PE:     A-accumulation MMs, numer (N=128) + den (N=4) (~30us)
      DMA:    16.9 MB bf16 at ~390 GB/s                     (~43us)
  - Normalize: one tensor_tensor per 2 q-macros over a 2-bank PSUM tile
    [128, 1024] with a stride-0 broadcast reciprocal operand (1x mode is
    forced by the f32 PSUM read anyway, so nothing is lost to broadcast).
  - Reciprocal via reciprocal_approx_fast (~18 correct bits, den ~1e5 so
    EPS=1e-6 is a 1e-11 perturbation and is dropped).
  - Junk-MM bursts at kernel start and at the two group barriers keep the
    PE HAM clock gate at 8/8 where it matters.
"""

import sys

for _p in ("/opt/trn_rl_repo",):
    if _p not in sys.path:
        sys.path.insert(0, _p)

import ml_dtypes
import numpy as np

from concourse import bacc, bass, mybir, tile
from concourse.bass_utils import run_bass_kernel_spmd

# ---------------------------------------------------------------- constants
N_BATCH = 8
L = 8192
S = 8192
H = 8
D = 32
P = 128

F32 = mybir.dt.float32
BF16 = mybir.dt.bfloat16
AF = mybir.ActivationFunctionType
OP = mybir.AluOpType

G = 2          # head groups (4 heads each; 4*32 = 128 partitions)
NM = 8         # K/V s-macros per group (1024 s-rows each)
MB = 8         # 128-row s-subtiles per macro
VA = P + 1     # 129: V group columns + ones column
KCOLS = MB * P         # 1024
VCOLS = MB * VA        # 1032
KVCOLS = KCOLS + VCOLS  # 2056
NDP = 4        # Q double-pairs per group (2048 l-columns each)
QCOLS = 2048


def _bcast_last(ap, n):
    """Append a stride-0 dim of size n to an AP (free-dim broadcast)."""
    ap = ap.unsqueeze(ap.ndim)
    return ap.broadcast_to(tuple(ap.shape[:-1]) + (n,))


def _build_body(nc, tc, qq, kv, og):
    with (
        tc.tile_pool(name="iokv", bufs=5) as iokv,
        tc.tile_pool(name="ioq", bufs=3) as ioq,
        tc.tile_pool(name="ewk", bufs=2) as ewk,
        tc.tile_pool(name="ewq", bufs=2) as ewq,
        tc.tile_pool(name="qp", bufs=1) as qp,
        tc.tile_pool(name="misc", bufs=1) as misc,
        tc.tile_pool(name="small", bufs=2) as small,
        tc.tile_pool(name="outp", bufs=2) as outp,
        tc.tile_pool(name="pacc", bufs=1, space="PSUM") as paccp,
        tc.tile_pool(name="psn", bufs=2, space="PSUM") as psn,
        tc.tile_pool(name="psd", bufs=1, space="PSUM") as psd,
    ):
        pacc = [
            paccp.tile([P, 512], F32, tag=f"pacc{g}", name=f"pacc{g}")
            for g in range(G)
        ]
        phiq = {}
        amat = {}
        bmat = {}

        # HAM warm-up: dense dummy matmuls while the first DMAs prefill.
        wz = misc.tile([P, 512], BF16, tag="warm", name="warm")
        nc.vector.memset(wz[:], 0.0)
        for _ in range(9):
            nc.tensor.matmul(
                pacc[0][:], wz[:, 0:P], wz[:], start=True, stop=True
            )

        def a_macro(g, m):
            kvt = iokv.tile([P, KVCOLS], BF16, tag="kv")
            nc.sync.dma_start(kvt[:], kv[g, m])
            kpart = kvt[:, 0:KCOLS]
            e = ewk.tile([P, KCOLS], BF16, tag="ke")
            t = ewk.tile([P, KCOLS], BF16, tag="kt")
            ph = ewk.tile([P, KCOLS], BF16, tag="kphi")
            nc.scalar.activation(e[:], kpart, AF.Exp)
            nc.vector.tensor_scalar(t[:], kpart, 0.0, 1.0, OP.max, OP.add)
            nc.vector.tensor_tensor(ph[:], e[:], t[:], OP.min)
            for b in range(MB):
                nc.tensor.matmul(
                    pacc[g][:, 0:VA],
                    ph[:, b * P : (b + 1) * P],
                    kvt[:, KCOLS + b * VA : KCOLS + (b + 1) * VA],
                    start=(m == 0 and b == 0),
                    stop=(m == NM - 1 and b == MB - 1),
                )

        def qprep(g, dp):
            qt = ioq.tile([P, QCOLS], BF16, tag="qt")
            nc.sync.dma_start(qt[:], qq[g, dp])
            e = ewq.tile([P, QCOLS], BF16, tag="qe")
            t = ewq.tile([P, QCOLS], BF16, tag="qt2")
            ph = qp.tile([P, QCOLS], BF16, tag=f"phiq{g}_{dp}",
                         name=f"phiq{g}_{dp}")
            nc.scalar.activation(e[:], qt[:], AF.Exp)
            nc.vector.tensor_scalar(t[:], qt[:], 0.0, 1.0, OP.max, OP.add)
            nc.vector.tensor_tensor(ph[:], e[:], t[:], OP.min)
            phiq[(g, dp)] = ph

        def assemble(g):
            am = misc.tile([P, P], BF16, tag=f"am{g}", name=f"am{g}")
            bm = misc.tile([P, 4], BF16, tag=f"bm{g}", name=f"bm{g}")
            nc.vector.memset(am[:], 0.0)
            nc.vector.memset(bm[:], 0.0)
            for j in range(4):
                r0 = 32 * j
                nc.vector.tensor_scalar(
                    am[r0 : r0 + 32, r0 : r0 + 32],
                    pacc[g][r0 : r0 + 32, r0 : r0 + 32],
                    0.0, None, OP.add,
                )
                nc.vector.tensor_scalar(
                    bm[r0 : r0 + 32, j : j + 1],
                    pacc[g][r0 : r0 + 32, P : P + 1],
                    0.0, None, OP.add,
                )
            amat[g] = am
            bmat[g] = bm

        # state shared across a double-pair (two b_pair calls)
        dpstate = {}

        def b_pair(g, mp):
            """Query pass for one pair of q-macros (1024 l-rows)."""
            half = mp % 2
            if half == 0:
                dpstate["dn"] = psd.tile([P, 64], F32, tag="dn", name="dn")
                dpstate["ot"] = outp.tile([P, 2 * 1024], BF16, tag="ot", name="ot")
            dn = dpstate["dn"]
            ot = dpstate["ot"]
            nm = psn.tile([P, 1024], F32, tag="nm")
            ph = phiq[(g, mp // 2)]
            for qs in range(8):  # (qmacro-in-pair, subtile)
                w = ph[:, (half * 8 + qs) * P : (half * 8 + qs + 1) * P]
                nc.tensor.matmul(
                    nm[:, qs * P : (qs + 1) * P], w, amat[g][:],
                    start=True, stop=True,
                )
                nc.tensor.matmul(
                    dn[:, half * 32 + qs * 4 : half * 32 + (qs + 1) * 4],
                    w, bmat[g][:], start=True, stop=True,
                )
            rcp = small.tile([P, 32], F32, tag="rcp")
            nc.vector.reciprocal_approx_fast(
                out=rcp[:], in_=dn[:, half * 32 : half * 32 + 32]
            )
            nc.vector.tensor_tensor(
                ot[:, half * 1024 : (half + 1) * 1024].rearrange(
                    "p (qs j c) -> p qs j c", qs=8, j=4, c=32
                ),
                nm[:].rearrange("p (qs j c) -> p qs j c", qs=8, j=4, c=32),
                _bcast_last(
                    rcp[:].rearrange("p (qs j) -> p qs j", qs=8, j=4), 32
                ),
                OP.mult,
            )
            if half == 1:
                # output DMA on the gpsimd queue: never blocks input stream
                nc.gpsimd.dma_start(og[g, mp // 2], ot[:])

        # -------- group 0: A/b accumulation + Q prep (both groups) ----------
        for m in range(NM):
            a_macro(0, m)
            if m % 2 == 0:
                qprep(0, m // 2)
            else:
                qprep(1, m // 2)
        assemble(0)

        # -------- group 1 accumulation overlapped with group 0 queries ------
        for m in range(NM):
            a_macro(1, m)
            b_pair(0, m)
        assemble(1)

        # ---------------- group 1 queries (tail) ----------------
        for mp in range(2 * NDP):
            b_pair(1, mp)


_NC_CACHE = None


def build_nc():
    global _NC_CACHE
    if _NC_CACHE is not None:
        return _NC_CACHE
    nc = bacc.Bacc(
        "TRN2",
        target_bir_lowering=False,
        debug=False,
        enable_asserts=False,
        num_devices=N_BATCH,
    )
    qq = nc.dram_tensor("qq", [G, NDP, P, QCOLS], BF16, kind="ExternalInput").ap()
    kv = nc.dram_tensor("kv", [G, NM, P, KVCOLS], BF16, kind="ExternalInput").ap()
    og = nc.dram_tensor("og", [G, NDP, P, 2 * 1024], BF16, kind="ExternalOutput").ap()
    with tile.TileContext(nc) as tc:
        _build_body(nc, tc, qq, kv, og)
    nc.compile()
    _NC_CACHE = nc
    return nc


def make_in_maps(queries, keys, values):
    queries = np.asarray(queries, dtype=np.float32)
    keys = np.asarray(keys, dtype=np.float32)
    values = np.asarray(values, dtype=np.float32)
    bf = ml_dtypes.bfloat16
    in_maps = []
    for n in range(N_BATCH):
        kvn = np.empty((G, NM, P, KVCOLS), dtype=bf)
        qqn = np.empty((G, NDP, P, QCOLS), dtype=bf)
        for g in range(G):
            # K group slab: [m][p][(b, jd)]
            Kg = keys[n][:, 4 * g : 4 * g + 4, :].reshape(S, P)
            kvn[g, :, :, 0:KCOLS] = (
                Kg.reshape(NM, MB, P, P).transpose(0, 2, 1, 3)
                .reshape(NM, P, KCOLS).astype(bf)
            )
            # V group slab with ones column: [m][p][(b, v|1)]
            Vg = values[n][:, 4 * g : 4 * g + 4, :].reshape(S, P)
            V1 = np.ones((S, VA), dtype=np.float32)
            V1[:, 0:P] = Vg
            kvn[g, :, :, KCOLS:] = (
                V1.reshape(NM, MB, P, VA).transpose(0, 2, 1, 3)
                .reshape(NM, P, VCOLS).astype(bf)
            )
            # Q transposed group-major: [dp][jd, l]
            Qg = queries[n][:, 4 * g : 4 * g + 4, :].reshape(L, P)
            qqn[g] = (
                Qg.T.reshape(P, NDP, QCOLS).transpose(1, 0, 2).astype(bf)
            )
        in_maps.append({"qq": qqn, "kv": kvn})
    return in_maps


def run(queries, keys, values, trace=False, **kwargs):
    nc = build_nc()
    in_maps = make_in_maps(queries, keys, values)
    res = run_bass_kernel_spmd(
        nc, in_maps, core_ids=list(range(N_BATCH)), trace=trace, **kwargs
    )
    outs = []
    for n in range(N_BATCH):
        o = res.results[n]["og"].astype(np.float32)
        # og[g, dp, p, (mp2, q, s, j, v)]; l = (((dp*2+mp2)*2+q)*4+s)*128+p
        o = o.reshape(G, NDP, P, 2, 2, 4, 4, 32)
        o = o.transpose(1, 3, 4, 5, 2, 0, 6, 7).reshape(L, H, D)
        outs.append(o)
    return np.stack(outs, axis=0), res


def kernel(queries, keys, values):
    out, _ = run(queries, keys, values, trace=False)
    return out


# revision 7
# speedup vs baseline: 1.0783x; 1.0084x over previous
"""Linear attention ("Transformers are RNNs") on 8 Trainium2 NeuronCores.

Problem: N=8, L=S=8192, H=8, D=Dv=32, f32.
    phi(x) = elu(x)+1
    A[d,v] = sum_s phi(K)[s,d] V[s,v]     (the /v_length ... *v_length cancels)
    b[d]   = sum_s phi(K)[s,d]
    out[l,v] = (sum_d phi(Q)[l,d] A[d,v]) / (sum_d phi(Q)[l,d] b[d] + EPS)

Sharding: batch element n -> core n (fully independent, no collectives).

v4 design — single continuous DMA-bound stream, group-pipelined:
  - phi via the exact identity  phi(x) = max(min(e^x, 1), x+1):
    for x>=0 min(e^x,1)=1 so the max yields 1+x; for x<0, e^x >= x+1
    always, so the max yields e^x.  The host ships x+1 (bf16), the ACT
    engine computes e^x = Exp((x+1) - 1) via its bias input, and phi is a
    SINGLE DVE scalar_tensor_tensor: (e min 1.0) max (x+1), running in
    2x packed mode.  This halves the DVE cost of phi vs the naive
    max/add + min pair and keeps ScalarE at one pass per element.
  - Heads split into G=2 groups of 4 (linear attention is separable per
    head).  K/V stream group-major, so group 0's A/b finish at the
    half-way point of the input stream and group 0's entire query pass
    (matmuls, reciprocal, normalize, output DMA) overlaps group 1's K/V
    accumulation.  Only group 1's query pass sits in the tail.
  - All DMAs are large contiguous slabs: 8x K|V slab-pairs (1MB), 8x Q
    (512KB), 8x out (512KB).  Input DMAs on the sync queue in stream
    order; output DMAs on the gpsimd queue so a not-yet-ready output
    never head-of-line-blocks the input stream.
  - Normalize: one tensor_tensor per 2 q-macros over a 2-bank PSUM tile
    [128, 1024] with a stride-0 broadcast reciprocal operand (1x mode is
    forced by the f32 PSUM read anyway, so broadcast costs nothing).
  - reciprocal_approx_fast batched over 4 q-macros (~18 correct bits,
    den ~1e5 so EPS=1e-6 is a 1e-11 perturbation and is dropped).
  - Engine totals (errata-adjusted model): DMA ~43us at ~390GB/s,
    DVE ~40us, ACT ~34us, PE pipelined ~106ns/MM issue rate.
"""

import sys

for _p in ("/opt/trn_rl_repo",):
    if _p not in sys.path:
        sys.path.insert(0, _p)

import ml_dtypes
import numpy as np

from concourse import bacc, bass, mybir, tile
from concourse.bass_utils import run_bass_kernel_spmd

# ---------------------------------------------------------------- constants
N_BATCH = 8
L = 8192
S = 8192
H = 8
D = 32
P = 128

F32 = mybir.dt.float32
BF16 = mybir.dt.bfloat16
AF = mybir.ActivationFunctionType
OP = mybir.AluOpType

G = 2          # head groups (4 heads each; 4*32 = 128 partitions)
NMP = 4        # K/V slab-pairs per group (2048 s-rows each)
MB = 16        # 128-row s-subtiles per slab-pair
VA = P + 1     # 129: V group columns + ones column
SLAB = 2056    # one old slab: 8*128 K cols + 8*129 V cols
KVCOLS = 2 * SLAB  # 4112
NDP = 4        # Q double-pairs per group (2048 l-columns each)
QCOLS = 2048


def _bcast_last(ap, n):
    """Append a stride-0 dim of size n to an AP (free-dim broadcast)."""
    ap = ap.unsqueeze(ap.ndim)
    return ap.broadcast_to(tuple(ap.shape[:-1]) + (n,))


def _build_body(nc, tc, qq, kv, og):
    with (
        tc.tile_pool(name="iokv", bufs=3) as iokv,
        tc.tile_pool(name="ioq", bufs=3) as ioq,
        tc.tile_pool(name="ewk", bufs=2) as ewk,
        tc.tile_pool(name="ewq", bufs=2) as ewq,
        tc.tile_pool(name="qp", bufs=1) as qp,
        tc.tile_pool(name="misc", bufs=1) as misc,
        tc.tile_pool(name="small", bufs=2) as small,
        tc.tile_pool(name="outp", bufs=2) as outp,
        tc.tile_pool(name="pacc", bufs=1, space="PSUM") as paccp,
        tc.tile_pool(name="psn", bufs=2, space="PSUM") as psn,
        tc.tile_pool(name="psd", bufs=1, space="PSUM") as psd,
    ):
        pacc = [
            paccp.tile([P, 512], F32, tag=f"pacc{g}", name=f"pacc{g}")
            for g in range(G)
        ]
        phiq = {}
        amat = {}
        bmat = {}

        # bias column for exp((x+1) - 1)
        nbias = misc.tile([P, 1], F32, tag="nbias", name="nbias")
        nc.gpsimd.memset(nbias[:], -1.0)

        # HAM warm-up: dense dummy matmuls while the first DMAs prefill.
        wz = misc.tile([P, 512], BF16, tag="warm", name="warm")
        nc.gpsimd.memset(wz[:], 0.0)
        for _ in range(9):
            nc.tensor.matmul(
                pacc[0][:], wz[:, 0:P], wz[:], start=True, stop=True
            )

        def a_macro(g, mp2):
            """One K|V slab-pair (2048 s-rows) of group g."""
            kvt = iokv.tile([P, KVCOLS], BF16, tag="kv")
            nc.sync.dma_start(kvt[:], kv[g, mp2])
            # K+1 part: two 1024-col runs at offsets 0 and SLAB
            kp1 = kvt[:].rearrange("p (s c) -> p s c", s=2, c=SLAB)[:, :, 0:1024]
            e = ewk.tile([P, 2048], BF16, tag="ke")
            ph = ewk.tile([P, 2048], BF16, tag="kphi")
            e2 = e[:].rearrange("p (s c) -> p s c", s=2)
            ph2 = ph[:].rearrange("p (s c) -> p s c", s=2)
            # e = exp((x+1) - 1);  phi = (e min 1) max (x+1)
            nc.scalar.activation(e2, kp1, AF.Exp, bias=nbias[:])
            nc.vector.scalar_tensor_tensor(ph2, e2, 1.0, kp1, OP.min, OP.max)
            first = mp2 == 0
            last = mp2 == NMP - 1
            for b in range(MB):
                voff = (b // 8) * SLAB + 1024 + (b % 8) * VA
                nc.tensor.matmul(
                    pacc[g][:, 0:VA],
                    ph[:, b * P : (b + 1) * P],
                    kvt[:, voff : voff + VA],
                    start=(first and b == 0),
                    stop=(last and b == MB - 1),
                )

        def qprep(g, dp):
            qt = ioq.tile([P, QCOLS], BF16, tag="qt")
            nc.sync.dma_start(qt[:], qq[g, dp])
            e = ewq.tile([P, QCOLS], BF16, tag="qe")
            ph = qp.tile([P, QCOLS], BF16, tag=f"phiq{g}_{dp}",
                         name=f"phiq{g}_{dp}")
            nc.scalar.activation(e[:], qt[:], AF.Exp, bias=nbias[:])
            nc.vector.scalar_tensor_tensor(ph[:], e[:], 1.0, qt[:], OP.min, OP.max)
            phiq[(g, dp)] = ph

        def assemble(g):
            am = misc.tile([P, P], BF16, tag=f"am{g}", name=f"am{g}")
            bm = misc.tile([P, 4], BF16, tag=f"bm{g}", name=f"bm{g}")
            nc.vector.memset(am[:], 0.0)
            nc.vector.memset(bm[:], 0.0)
            for j in range(4):
                r0 = 32 * j
                nc.scalar.copy(
                    am[r0 : r0 + 32, r0 : r0 + 32],
                    pacc[g][r0 : r0 + 32, r0 : r0 + 32],
                )
                nc.scalar.copy(
                    bm[r0 : r0 + 32, j : j + 1],
                    pacc[g][r0 : r0 + 32, P : P + 1],
                )
            amat[g] = am
            bmat[g] = bm

        # state shared across a double-pair (two b_pair calls)
        dpstate = {}

        def b_pair(g, mp):
            """Query pass for one pair of q-macros (1024 l-rows)."""
            half = mp % 2
            if half == 0:
                dpstate["dn"] = psd.tile([P, 64], F32, tag="dn", name="dn")
                dpstate["ot"] = outp.tile([P, 2 * 1024], BF16, tag="ot", name="ot")
                dpstate["rcp"] = small.tile([P, 64], F32, tag="rcp", name="rcp")
            dn = dpstate["dn"]
            ot = dpstate["ot"]
            rcp = dpstate["rcp"]
            nm = psn.tile([P, 1024], F32, tag="nm")
            ph = phiq[(g, mp // 2)]
            for qs in range(8):  # (qmacro-in-pair, subtile)
                w = ph[:, (half * 8 + qs) * P : (half * 8 + qs + 1) * P]
                nc.tensor.matmul(
                    nm[:, qs * P : (qs + 1) * P], w, amat[g][:],
                    start=True, stop=True,
                )
                nc.tensor.matmul(
                    dn[:, half * 32 + qs * 4 : half * 32 + (qs + 1) * 4],
                    w, bmat[g][:], start=True, stop=True,
                )
            # one reciprocal per double-pair, after the second half's den MMs
            nc.vector.reciprocal_approx_fast(
                out=rcp[:, half * 32 : half * 32 + 32],
                in_=dn[:, half * 32 : half * 32 + 32],
            )
            nc.vector.tensor_tensor(
                ot[:, half * 1024 : (half + 1) * 1024].rearrange(
                    "p (qs j c) -> p qs j c", qs=8, j=4, c=32
                ),
                nm[:].rearrange("p (qs j c) -> p qs j c", qs=8, j=4, c=32),
                _bcast_last(
                    rcp[:, half * 32 : half * 32 + 32].rearrange(
                        "p (qs j) -> p qs j", qs=8, j=4
                    ),
                    32,
                ),
                OP.mult,
            )
            if half == 1:
                # output DMA on the gpsimd queue: never blocks input stream
                nc.gpsimd.dma_start(og[g, mp // 2], ot[:])

        # -------- group 0: A/b accumulation + Q prep (both groups) ----------
        for mp2 in range(NMP):
            a_macro(0, mp2)
            qprep(0, mp2)
            qprep(1, mp2)
        assemble(0)

        # -------- group 1 accumulation overlapped with group 0 queries ------
        for mp2 in range(NMP):
            a_macro(1, mp2)
            b_pair(0, 2 * mp2)
            b_pair(0, 2 * mp2 + 1)
        assemble(1)

        # ---------------- group 1 queries (tail) ----------------
        for mp in range(2 * NDP):
            b_pair(1, mp)


_NC_CACHE = None


def build_nc():
    global _NC_CACHE
    if _NC_CACHE is not None:
        return _NC_CACHE
    nc = bacc.Bacc(
        "TRN2",
        target_bir_lowering=False,
        debug=False,
        enable_asserts=False,
        num_devices=N_BATCH,
    )
    qq = nc.dram_tensor("qq", [G, NDP, P, QCOLS], BF16, kind="ExternalInput").ap()
    kv = nc.dram_tensor("kv", [G, NMP, P, KVCOLS], BF16, kind="ExternalInput").ap()
    og = nc.dram_tensor("og", [G, NDP, P, 2 * 1024], BF16, kind="ExternalOutput").ap()
    with tile.TileContext(nc) as tc:
        _build_body(nc, tc, qq, kv, og)
    nc.compile()
    _NC_CACHE = nc
    return nc


def make_in_maps(queries, keys, values):
    queries = np.asarray(queries, dtype=np.float32)
    keys = np.asarray(keys, dtype=np.float32)
    values = np.asarray(values, dtype=np.float32)
    bf = ml_dtypes.bfloat16
    in_maps = []
    for n in range(N_BATCH):
        kvn = np.empty((G, 8, P, SLAB), dtype=bf)
        qqn = np.empty((G, NDP, P, QCOLS), dtype=bf)
        for g in range(G):
            # K group slab (shifted by +1 for the bias-exp trick)
            Kg = keys[n][:, 4 * g : 4 * g + 4, :].reshape(S, P) + 1.0
            kvn[g, :, :, 0:1024] = (
                Kg.reshape(8, 8, P, P).transpose(0, 2, 1, 3)
                .reshape(8, P, 1024).astype(bf)
            )
            # V group slab with ones column
            Vg = values[n][:, 4 * g : 4 * g + 4, :].reshape(S, P)
            V1 = np.ones((S, VA), dtype=np.float32)
            V1[:, 0:P] = Vg
            kvn[g, :, :, 1024:] = (
                V1.reshape(8, 8, P, VA).transpose(0, 2, 1, 3)
                .reshape(8, P, 8 * VA).astype(bf)
            )
            # Q+1 transposed group-major: [dp][jd, l]
            Qg = queries[n][:, 4 * g : 4 * g + 4, :].reshape(L, P) + 1.0
            qqn[g] = (
                Qg.T.reshape(P, NDP, QCOLS).transpose(1, 0, 2).astype(bf)
            )
        # pair adjacent slabs: [g, 4, p, 2*SLAB]
        kvp = np.ascontiguousarray(
            kvn.reshape(G, NMP, 2, P, SLAB).transpose(0, 1, 3, 2, 4)
            .reshape(G, NMP, P, KVCOLS)
        )
        in_maps.append({"qq": qqn, "kv": kvp})
    return in_maps


def run(queries, keys, values, trace=False, **kwargs):
    nc = build_nc()
    in_maps = make_in_maps(queries, keys, values)
    res = run_bass_kernel_spmd(
        nc, in_maps, core_ids=list(range(N_BATCH)), trace=trace, **kwargs
    )
    outs = []
    for n in range(N_BATCH):
        o = res.results[n]["og"].astype(np.float32)
        # og[g, dp, p, (mp2, q, s, j, v)]; l = (((dp*2+mp2)*2+q)*4+s)*128+p
        o = o.reshape(G, NDP, P, 2, 2, 4, 4, 32)
        o = o.transpose(1, 3, 4, 5, 2, 0, 6, 7).reshape(L, H, D)
        outs.append(o)
    return np.stack(outs, axis=0), res


def kernel(queries, keys, values):
    out, _ = run(queries, keys, values, trace=False)
    return out


# revision 8
# speedup vs baseline: 1.1556x; 1.0718x over previous
"""Linear attention ("Transformers are RNNs") on 8 Trainium2 NeuronCores.

Problem: N=8, L=S=8192, H=8, D=Dv=32, f32.
    phi(x) = elu(x)+1
    A[d,v] = sum_s phi(K)[s,d] V[s,v]     (the /v_length ... *v_length cancels)
    b[d]   = sum_s phi(K)[s,d]
    out[l,v] = (sum_d phi(Q)[l,d] A[d,v]) / (sum_d phi(Q)[l,d] b[d] + EPS)

Sharding: batch element n -> core n (fully independent, no collectives).

v4 design — single continuous DMA-bound stream, group-pipelined:
  - phi via the exact identity  phi(x) = max(min(e^x, 1), x+1):
    for x>=0 min(e^x,1)=1 so the max yields 1+x; for x<0, e^x >= x+1
    always, so the max yields e^x.  The host ships x+1 (bf16), the ACT
    engine computes e^x = Exp((x+1) - 1) via its bias input, and phi is a
    SINGLE DVE scalar_tensor_tensor: (e min 1.0) max (x+1), running in
    2x packed mode.  This halves the DVE cost of phi vs the naive
    max/add + min pair and keeps ScalarE at one pass per element.
  - Heads split into G=2 groups of 4 (linear attention is separable per
    head).  K/V stream group-major, so group 0's A/b finish at the
    half-way point of the input stream and group 0's entire query pass
    (matmuls, reciprocal, normalize, output DMA) overlaps group 1's K/V
    accumulation.  Only group 1's query pass sits in the tail.
  - All DMAs are large contiguous slabs: 8x K|V slab-pairs (1MB), 8x Q
    (512KB), 8x out (512KB).  Input DMAs on the sync queue in stream
    order; output DMAs on the gpsimd queue so a not-yet-ready output
    never head-of-line-blocks the input stream.
  - Normalize: one tensor_tensor per 2 q-macros over a 2-bank PSUM tile
    [128, 1024] with a stride-0 broadcast reciprocal operand (1x mode is
    forced by the f32 PSUM read anyway, so broadcast costs nothing).
  - reciprocal_approx_fast batched over 4 q-macros (~18 correct bits,
    den ~1e5 so EPS=1e-6 is a 1e-11 perturbation and is dropped).
  - Engine totals (errata-adjusted model): DMA ~43us at ~390GB/s,
    DVE ~40us, ACT ~34us, PE pipelined ~106ns/MM issue rate.
"""

import sys

for _p in ("/opt/trn_rl_repo",):
    if _p not in sys.path:
        sys.path.insert(0, _p)

import ml_dtypes
import numpy as np

from concourse import bacc, bass, mybir, tile
from concourse.bass_utils import run_bass_kernel_spmd

# ---------------------------------------------------------------- constants
N_BATCH = 8
L = 8192
S = 8192
H = 8
D = 32
P = 128

F32 = mybir.dt.float32
BF16 = mybir.dt.bfloat16
AF = mybir.ActivationFunctionType
OP = mybir.AluOpType

G = 2          # head groups (4 heads each; 4*32 = 128 partitions)
NMP = 4        # K/V slab-pairs per group (2048 s-rows each)
MB = 16        # 128-row s-subtiles per slab-pair
VA = P + 1     # 129: V group columns + ones column
SLAB = 2056    # one old slab: 8*128 K cols + 8*129 V cols
KVCOLS = 2 * SLAB  # 4112
NDP = 4        # Q double-pairs per group (2048 l-columns each)
QCOLS = 2048


def _bcast_last(ap, n):
    """Append a stride-0 dim of size n to an AP (free-dim broadcast)."""
    ap = ap.unsqueeze(ap.ndim)
    return ap.broadcast_to(tuple(ap.shape[:-1]) + (n,))


def _build_body(nc, tc, qq, kv, og):
    with (
        tc.tile_pool(name="iokv", bufs=4) as iokv,
        tc.tile_pool(name="ioq", bufs=6) as ioq,
        tc.tile_pool(name="ewk", bufs=3) as ewk,
        tc.tile_pool(name="ewq", bufs=3) as ewq,
        tc.tile_pool(name="qp", bufs=1) as qp,
        tc.tile_pool(name="misc", bufs=1) as misc,
        tc.tile_pool(name="small", bufs=2) as small,
        tc.tile_pool(name="outp", bufs=2) as outp,
        tc.tile_pool(name="pacc", bufs=1, space="PSUM") as paccp,
        tc.tile_pool(name="psn", bufs=3, space="PSUM") as psn,
        tc.tile_pool(name="psd", bufs=1, space="PSUM") as psd,
    ):
        pacc = {}
        phiq = {}
        amat = {}
        bmat = {}

        # bias column for exp((x+1) - 1)
        nbias = misc.tile([P, 1], F32, tag="nbias", name="nbias")
        nc.gpsimd.memset(nbias[:], -1.0)

        # HAM warm-up: dense dummy matmuls while the first DMAs prefill.
        wz = misc.tile([P, 512], BF16, tag="warm", name="warm")
        nc.gpsimd.memset(wz[:], 0.0)
        pacc[0] = paccp.tile([P, 512], F32, tag="pacc", name="pacc")
        for _ in range(9):
            nc.tensor.matmul(
                pacc[0][:], wz[:, 0:P], wz[:], start=True, stop=True
            )

        def a_macro(g, mp2):
            """One K|V slab-pair (2048 s-rows) of group g."""
            if mp2 == 0 and g > 0:
                pacc[g] = paccp.tile([P, 512], F32, tag="pacc", name="pacc")
            kvt = iokv.tile([P, KVCOLS], BF16, tag="kv")
            nc.sync.dma_start(kvt[:], kv[g, mp2])
            # K+1 part: two 1024-col runs at offsets 0 and SLAB
            kp1 = kvt[:].rearrange("p (s c) -> p s c", s=2, c=SLAB)[:, :, 0:1024]
            e = ewk.tile([P, 2048], BF16, tag="ke")
            ph = ewk.tile([P, 2048], BF16, tag="kphi")
            t = ewk.tile([P, 2048], BF16, tag="kt")
            e2 = e[:].rearrange("p (s c) -> p s c", s=2)
            t2 = t[:].rearrange("p (s c) -> p s c", s=2)
            ph2 = ph[:].rearrange("p (s c) -> p s c", s=2)
            # e = exp((x+1) - 1);  t = max(x+1, 1);  phi = min(e, t)
            nc.scalar.activation(e2, kp1, AF.Exp, bias=nbias[:])
            nc.vector.tensor_scalar(t2, kp1, 1.0, None, OP.max)
            nc.vector.tensor_tensor(ph2, e2, t2, OP.min)
            first = mp2 == 0
            last = mp2 == NMP - 1
            for b in range(MB):
                voff = (b // 8) * SLAB + 1024 + (b % 8) * VA
                nc.tensor.matmul(
                    pacc[g][:, 0:VA],
                    ph[:, b * P : (b + 1) * P],
                    kvt[:, voff : voff + VA],
                    start=(first and b == 0),
                    stop=(last and b == MB - 1),
                )

        def qprep(g, dp):
            qt = ioq.tile([P, QCOLS], BF16, tag="qt")
            nc.sync.dma_start(qt[:], qq[g, dp])
            e = ewq.tile([P, QCOLS], BF16, tag="qe")
            ph = qp.tile([P, QCOLS], BF16, tag=f"phiq{g}_{dp}",
                         name=f"phiq{g}_{dp}")
            t = ewq.tile([P, QCOLS], BF16, tag="qt2")
            nc.scalar.activation(e[:], qt[:], AF.Exp, bias=nbias[:])
            nc.vector.tensor_scalar(t[:], qt[:], 1.0, None, OP.max)
            nc.vector.tensor_tensor(ph[:], e[:], t[:], OP.min)
            phiq[(g, dp)] = ph

        def assemble(g):
            am = misc.tile([P, P], BF16, tag=f"am{g}", name=f"am{g}")
            bm = misc.tile([P, 4], BF16, tag=f"bm{g}", name=f"bm{g}")
            nc.vector.memset(am[:], 0.0)
            nc.vector.memset(bm[:], 0.0)
            for j in range(4):
                r0 = 32 * j
                nc.scalar.copy(
                    am[r0 : r0 + 32, r0 : r0 + 32],
                    pacc[g][r0 : r0 + 32, r0 : r0 + 32],
                )
                nc.scalar.copy(
                    bm[r0 : r0 + 32, j : j + 1],
                    pacc[g][r0 : r0 + 32, P : P + 1],
                )
            amat[g] = am
            bmat[g] = bm

        # state shared across a double-pair (two b_pair calls)
        dpstate = {}

        def b_pair(g, mp):
            """Query pass for one pair of q-macros (1024 l-rows)."""
            half = mp % 2
            if half == 0:
                dpstate["dn"] = psd.tile([P, 64], F32, tag="dn", name="dn")
                dpstate["ot"] = outp.tile([P, 2 * 1024], BF16, tag="ot", name="ot")
                dpstate["rcp"] = small.tile([P, 64], F32, tag="rcp", name="rcp")
            dn = dpstate["dn"]
            ot = dpstate["ot"]
            rcp = dpstate["rcp"]
            nm = psn.tile([P, 1024], F32, tag="nm")
            ph = phiq[(g, mp // 2)]
            for qs in range(8):  # (qmacro-in-pair, subtile)
                w = ph[:, (half * 8 + qs) * P : (half * 8 + qs + 1) * P]
                nc.tensor.matmul(
                    nm[:, qs * P : (qs + 1) * P], w, amat[g][:],
                    start=True, stop=True,
                )
                nc.tensor.matmul(
                    dn[:, half * 32 + qs * 4 : half * 32 + (qs + 1) * 4],
                    w, bmat[g][:], start=True, stop=True,
                )
            # one reciprocal per double-pair, after the second half's den MMs
            nc.vector.reciprocal_approx_fast(
                out=rcp[:, half * 32 : half * 32 + 32],
                in_=dn[:, half * 32 : half * 32 + 32],
            )
            nc.vector.tensor_tensor(
                ot[:, half * 1024 : (half + 1) * 1024].rearrange(
                    "p (qs j c) -> p qs j c", qs=8, j=4, c=32
                ),
                nm[:].rearrange("p (qs j c) -> p qs j c", qs=8, j=4, c=32),
                _bcast_last(
                    rcp[:, half * 32 : half * 32 + 32].rearrange(
                        "p (qs j) -> p qs j", qs=8, j=4
                    ),
                    32,
                ),
                OP.mult,
            )
            if half == 1:
                # output DMA on the gpsimd queue: never blocks input stream
                nc.gpsimd.dma_start(og[g, mp // 2], ot[:])

        # -------- group 0: A/b accumulation + Q prep (both groups) ----------
        for mp2 in range(NMP):
            a_macro(0, mp2)
            qprep(0, mp2)
            qprep(1, mp2)
        assemble(0)

        # -------- group 1 accumulation overlapped with group 0 queries ------
        for mp2 in range(NMP):
            a_macro(1, mp2)
            b_pair(0, 2 * mp2)
            b_pair(0, 2 * mp2 + 1)
        assemble(1)

        # ---------------- group 1 queries (tail) ----------------
        for mp in range(2 * NDP):
            b_pair(1, mp)


_NC_CACHE = None


def build_nc():
    global _NC_CACHE
    if _NC_CACHE is not None:
        return _NC_CACHE
    nc = bacc.Bacc(
        "TRN2",
        target_bir_lowering=False,
        debug=False,
        enable_asserts=False,
        num_devices=N_BATCH,
    )
    qq = nc.dram_tensor("qq", [G, NDP, P, QCOLS], BF16, kind="ExternalInput").ap()
    kv = nc.dram_tensor("kv", [G, NMP, P, KVCOLS], BF16, kind="ExternalInput").ap()
    og = nc.dram_tensor("og", [G, NDP, P, 2 * 1024], BF16, kind="ExternalOutput").ap()
    with tile.TileContext(nc) as tc:
        _build_body(nc, tc, qq, kv, og)
    nc.compile()
    _NC_CACHE = nc
    return nc


def make_in_maps(queries, keys, values):
    queries = np.asarray(queries, dtype=np.float32)
    keys = np.asarray(keys, dtype=np.float32)
    values = np.asarray(values, dtype=np.float32)
    bf = ml_dtypes.bfloat16
    in_maps = []
    for n in range(N_BATCH):
        kvn = np.empty((G, 8, P, SLAB), dtype=bf)
        qqn = np.empty((G, NDP, P, QCOLS), dtype=bf)
        for g in range(G):
            # K group slab (shifted by +1 for the bias-exp trick)
            Kg = keys[n][:, 4 * g : 4 * g + 4, :].reshape(S, P) + 1.0
            kvn[g, :, :, 0:1024] = (
                Kg.reshape(8, 8, P, P).transpose(0, 2, 1, 3)
                .reshape(8, P, 1024).astype(bf)
            )
            # V group slab with ones column
            Vg = values[n][:, 4 * g : 4 * g + 4, :].reshape(S, P)
            V1 = np.ones((S, VA), dtype=np.float32)
            V1[:, 0:P] = Vg
            kvn[g, :, :, 1024:] = (
                V1.reshape(8, 8, P, VA).transpose(0, 2, 1, 3)
                .reshape(8, P, 8 * VA).astype(bf)
            )
            # Q+1 transposed group-major: [dp][jd, l]
            Qg = queries[n][:, 4 * g : 4 * g + 4, :].reshape(L, P) + 1.0
            qqn[g] = (
                Qg.T.reshape(P, NDP, QCOLS).transpose(1, 0, 2).astype(bf)
            )
        # pair adjacent slabs: [g, 4, p, 2*SLAB]
        kvp = np.ascontiguousarray(
            kvn.reshape(G, NMP, 2, P, SLAB).transpose(0, 1, 3, 2, 4)
            .reshape(G, NMP, P, KVCOLS)
        )
        in_maps.append({"qq": qqn, "kv": kvp})
    return in_maps


def run(queries, keys, values, trace=False, **kwargs):
    nc = build_nc()
    in_maps = make_in_maps(queries, keys, values)
    res = run_bass_kernel_spmd(
        nc, in_maps, core_ids=list(range(N_BATCH)), trace=trace, **kwargs
    )
    outs = []
    for n in range(N_BATCH):
        o = res.results[n]["og"].astype(np.float32)
        # og[g, dp, p, (mp2, q, s, j, v)]; l = (((dp*2+mp2)*2+q)*4+s)*128+p
        o = o.reshape(G, NDP, P, 2, 2, 4, 4, 32)
        o = o.transpose(1, 3, 4, 5, 2, 0, 6, 7).reshape(L, H, D)
        outs.append(o)
    return np.stack(outs, axis=0), res


def kernel(queries, keys, values):
    out, _ = run(queries, keys, values, trace=False)
    return out
